# revision 10
# baseline (speedup 1.0000x reference)
"""Bidirectional Mamba block on 8 Trainium2 NeuronCores.

Strategy
--------
Data-parallel over batch: each of the 8 cores runs one batch element
(both directions) end to end; no collectives.

Per core / per direction, with d_inner=512 split into 4 blocks of 128
partitions and the state dim n=16 split into 4 groups of 4:

  phase 1: x_conv = silu(conv1d(u @ in_w_x.T) + conv_b)  -- the depthwise
           causal conv is folded into the input projection on the PE by
           expanding the contraction dim to (4 taps x 256 d_model) with
           host-combined weights;  sz = silu(u @ in_w_z.T).
  phase 2: dbl = x_conv @ xproj.T (PE), delta = softplus(dt-proj + dt_b)
           (PE + one ACT Softplus pass). B/C rows of dbl bounce through a
           DRAM scratch so a stride-0-partition DMA can broadcast them to
           all 128 partitions.
  phase 3: per (block, n-group): a = exp(A[:,n] * delta) on ACT (scale is
           a per-partition AP, so the multiply is free), b = w * B_bcast,
           h = tensor_tensor_scan(a, b) (the native DVE linear-recurrence
           instruction, chained across t-chunks via per-partition initial
           values), y = sum_n h * C_bcast via a small add tree, then
           y = (y + x_conv * D) * sz, and the output projection on the PE
           (0.5 factor folded into out_w host-side).

The backward direction runs first on a host-reversed copy of the input;
its output (y_T) is stored to DRAM and combined, reversed, with the
forward direction's output projection.
"""

import contextlib
import functools
import sys

for _p in ("/opt/trn_rl_repo",):
    if _p not in sys.path:
        sys.path.insert(0, _p)

import numpy as np

import concourse.bass as bass
import concourse.bacc as bacc
import concourse.mybir as mybir
import concourse.tile as tile

F16 = mybir.dt.float16
F32 = mybir.dt.float32
AOP = mybir.AluOpType
ACT = mybir.ActivationFunctionType

D_MODEL = 256
D_INNER = 512
NSTATE = 16
DT_RANK = 16
KCONV = 4
NBLK = D_INNER // 128   # 4 d_inner blocks
NMT = D_MODEL // 128    # 2 d_model tiles
NJ = 4                  # n per group
NG = NSTATE // NJ       # 4 groups
N_CORES = 8

L_FULL = 4096
T_FULL = 512

# Native Silu ACT function exists on hardware but is not implemented by
# CoreSim; sim_test flips this to False to use sigmoid+multiply instead.
SILU_NATIVE = True


def _emit_dir(nc, pools, prm, dirn, L, T, ob, y_param):
    nchunk = L // T
    wp, fl, core, ps, dram, sm = (
        pools["weights"], pools["fullL"], pools["core"],
        pools["psum"], pools["dram"], pools["small"],
    )

    def dma(out, in_):
        nc.sync.dma_start(out=out, in_=in_)

    # ---- weights into SBUF ----
    wx = []
    for i in range(2 * KCONV):
        t = wp.tile([128, D_INNER], F16, tag=f"wx{i}", name=f"wx{i}")
        dma(t, prm[f"{dirn}_wx"][i])
        wx.append(t)
    wz = []
    for i in range(2):
        t = wp.tile([128, D_INNER], F16, tag=f"wz{i}", name=f"wz{i}")
        dma(t, prm[f"{dirn}_wz"][i])
        wz.append(t)
    wxp = []
    for b in range(NBLK):
        t = wp.tile([128, DT_RANK + 2 * NSTATE], F16, tag=f"wxp{b}", name=f"wxp{b}")
        dma(t, prm[f"{dirn}_wxp"][b])
        wxp.append(t)
    wdt = wp.tile([DT_RANK, D_INNER], F16, tag="wdt", name="wdt")
    dma(wdt, prm[f"{dirn}_wdt"][:])
    wo = []
    for b in range(NBLK):
        t = wp.tile([128, D_MODEL], F16, tag=f"wo{b}", name=f"wo{b}")
        dma(t, prm[f"{dirn}_wo"][b])
        wo.append(t)
    cb, dtb, At, Dd = [], [], [], []
    for b in range(NBLK):
        t = wp.tile([128, 1], F32, tag=f"cb{b}", name=f"cb{b}")
        dma(t, prm[f"{dirn}_cb"][b])
        cb.append(t)
        t = wp.tile([128, 1], F32, tag=f"dtb{b}", name=f"dtb{b}")
        dma(t, prm[f"{dirn}_dtb"][b])
        dtb.append(t)
        t = wp.tile([128, NSTATE], F32, tag=f"A{b}", name=f"A{b}")
        dma(t, prm[f"{dirn}_A"][b])
        At.append(t)
        t = wp.tile([128, 1], F32, tag=f"D{b}", name=f"D{b}")
        dma(t, prm[f"{dirn}_D"][b])
        Dd.append(t)

    u_param = prm[f"u_{dirn}"]

    # ---- persistent per-direction tensors ----
    xc = [fl.tile([128, L], F16, tag=f"xc{b}", name=f"xc{b}") for b in range(NBLK)]
    szt = [fl.tile([128, L], F16, tag=f"sz{b}", name=f"sz{b}") for b in range(NBLK)]
    dl = [fl.tile([128, L], F16, tag=f"dl{b}", name=f"dl{b}") for b in range(NBLK)]
    state = [sm.tile([128, NSTATE], F32, tag=f"st{b}", name=f"st{b}") for b in range(NBLK)]
    bc = dram.tile([DT_RANK + 2 * NSTATE - DT_RANK, L], F16, tag="bc", name="bc")  # [32, L]

    # ---- phase 1: x_conv (conv folded into in-proj) and sz ----
    for c in range(nchunk):
        t0 = c * T
        u0 = core.tile([128, T + KCONV - 1], F16, tag="u0", name="u0")
        u1 = core.tile([128, T + KCONV - 1], F16, tag="u1", name="u1")
        dma(u0, u_param[0:128, t0:t0 + T + KCONV - 1])
        dma(u1, u_param[128:256, t0:t0 + T + KCONV - 1])
        uu = (u0, u1)
        for b in range(NBLK):
            px = ps.tile([128, T], F32, tag="px", name="px", bufs=2)
            for kb in range(2 * KCONV):
                k, dmb = divmod(kb, 2)
                nc.tensor.matmul(
                    px[:], wx[kb][:, b * 128:(b + 1) * 128],
                    uu[dmb][:, k:k + T],
                    start=(kb == 0), stop=(kb == 2 * KCONV - 1),
                )
            if SILU_NATIVE:
                nc.scalar.activation(
                    out=xc[b][:, t0:t0 + T], in_=px[:], func=ACT.Silu,
                    bias=cb[b][:], scale=1.0,
                )
            else:
                xq = core.tile([128, T], F32, tag="xq", name="xq")
                nc.vector.tensor_scalar(out=xq[:], in0=px[:], scalar1=cb[b][:],
                                        scalar2=None, op0=AOP.add)
                sg = core.tile([128, T], F32, tag="sg", name="sg")
                nc.scalar.activation(out=sg[:], in_=xq[:], func=ACT.Sigmoid)
                nc.vector.tensor_mul(xc[b][:, t0:t0 + T], xq[:], sg[:])
        for b in range(NBLK):
            pz = ps.tile([128, T], F32, tag="pz", name="pz")
            for dmb in range(2):
                nc.tensor.matmul(
                    pz[:], wz[dmb][:, b * 128:(b + 1) * 128],
                    uu[dmb][:, KCONV - 1:KCONV - 1 + T],
                    start=(dmb == 0), stop=(dmb == 1),
                )
            if SILU_NATIVE:
                nc.scalar.activation(out=szt[b][:, t0:t0 + T], in_=pz[:], func=ACT.Silu)
            else:
                sg2 = core.tile([128, T], F32, tag="sg2", name="sg2")
                nc.scalar.activation(out=sg2[:], in_=pz[:], func=ACT.Sigmoid)
                nc.vector.tensor_mul(szt[b][:, t0:t0 + T], pz[:], sg2[:])

    # ---- phase 2: dbl (dt/B/C) and delta ----
    for c in range(nchunk):
        t0 = c * T
        pd = ps.tile([DT_RANK + 2 * NSTATE, T], F32, tag="pd", name="pd")
        for b in range(NBLK):
            nc.tensor.matmul(
                pd[:], wxp[b][:], xc[b][:, t0:t0 + T],
                start=(b == 0), stop=(b == NBLK - 1),
            )
        dbl = core.tile([DT_RANK + 2 * NSTATE, T], F16, tag="dbl", name="dbl")
        nc.scalar.activation(out=dbl[:], in_=pd[:], func=ACT.Copy)
        dma(bc[:, t0:t0 + T], dbl[DT_RANK:, :])
        for b in range(NBLK):
            pt = ps.tile([128, T], F32, tag="pt", name="pt")
            nc.tensor.matmul(
                pt[:], wdt[:, b * 128:(b + 1) * 128], dbl[0:DT_RANK, :],
                start=True, stop=True,
            )
            et = core.tile([128, T], F32, tag="et", name="et")
            nc.scalar.activation(out=et[:], in_=pt[:], func=ACT.Exp,
                                 bias=dtb[b][:], scale=1.0)
            nc.scalar.activation(out=dl[b][:, t0:t0 + T], in_=et[:],
                                 func=ACT.Ln, bias=1.0, scale=1.0)

    # ---- phase 3: selective scan + gating + out-proj ----
    bc_ap = bc[:]
    for c in range(nchunk):
        t0 = c * T
        gts = []
        for b in range(NBLK):
            wch = core.tile([128, T], F16, tag="wch", name="wch")
            nc.vector.tensor_mul(wch[:], dl[b][:, t0:t0 + T], xc[b][:, t0:t0 + T])
            wch_ap = wch[:]
            wrep = bass.AP(
                tensor=wch_ap.tensor, offset=wch_ap.offset,
                ap=[wch_ap.ap[0], [0, NJ], [1, T]],
            )
            qroots = []
            for g in range(NG):
                Bb = core.tile([128, NJ, T], F16, tag="Bb", name="Bb")
                dma(Bb, bass.AP(
                    tensor=bc_ap.tensor,
                    offset=bc_ap.offset + (g * NJ) * bc_ap.ap[-2][0] + t0,
                    ap=[[0, 128], [bc_ap.ap[-2][0], NJ], [1, T]],
                ))
                Cb = core.tile([128, NJ, T], F16, tag="Cb", name="Cb")
                dma(Cb, bass.AP(
                    tensor=bc_ap.tensor,
                    offset=bc_ap.offset + (NSTATE + g * NJ) * bc_ap.ap[-2][0] + t0,
                    ap=[[0, 128], [bc_ap.ap[-2][0], NJ], [1, T]],
                ))
                at = core.tile([128, NJ, T], F16, tag="at", name="at")
                for j in range(NJ):
                    n = g * NJ + j
                    nc.scalar.activation(
                        out=at[:, j, :], in_=dl[b][:, t0:t0 + T],
                        func=ACT.Exp, scale=At[b][:, n:n + 1],
                    )
                bt = core.tile([128, NJ, T], F16, tag="bt", name="bt")
                nc.vector.tensor_mul(bt[:], wrep, Bb[:])
                ht = core.tile([128, NJ, T], F16, tag="ht", name="ht")
                for j in range(NJ):
                    n = g * NJ + j
                    init = 0.0 if c == 0 else state[b][:, n:n + 1]
                    nc.vector.tensor_tensor_scan(
                        ht[:, j, :], at[:, j, :], bt[:, j, :], init,
                        AOP.mult, AOP.add,
                    )
                nc.vector.tensor_copy(
                    out=state[b][:, g * NJ:(g + 1) * NJ], in_=ht[:, :, T - 1],
                )
                pt2 = core.tile([128, NJ, T], F16, tag="pt2", name="pt2")
                nc.vector.tensor_mul(pt2[:], ht[:], Cb[:])
                q1 = core.tile([128, 2, T], F16, tag="q1", name="q1")
                nc.vector.tensor_add(q1[:], pt2[:, 0:NJ:2, :], pt2[:, 1:NJ:2, :])
                qr = core.tile([128, T], F16, tag="qr", name="qr")
                nc.vector.tensor_add(qr[:], q1[:, 0, :], q1[:, 1, :])
                qroots.append(qr)
            q01 = core.tile([128, T], F16, tag="q01", name="q01")
            nc.vector.tensor_add(q01[:], qroots[0][:], qroots[1][:])
            q23 = core.tile([128, T], F16, tag="q23", name="q23")
            nc.vector.tensor_add(q23[:], qroots[2][:], qroots[3][:])
            y2 = core.tile([128, T], F16, tag="y2", name="y2")
            nc.vector.tensor_add(y2[:], q01[:], q23[:])
            y1 = core.tile([128, T], F16, tag="y1", name="y1")
            nc.vector.scalar_tensor_tensor(
                out=y1[:], in0=xc[b][:, t0:t0 + T], scalar=Dd[b][:],
                in1=y2[:], op0=AOP.mult, op1=AOP.add,
            )
            gt = core.tile([128, T], F16, tag=f"gt{b}", name=f"gt{b}")
            nc.vector.tensor_mul(gt[:], y1[:], szt[b][:, t0:t0 + T])
            gts.append(gt)
        for mt in range(NMT):
            po = ps.tile([128, T], F32, tag="po", name="po", bufs=2)
            for b in range(NBLK):
                nc.tensor.matmul(
                    po[:], wo[b][:, mt * 128:(mt + 1) * 128], gts[b][:],
                    start=(b == 0), stop=(b == NBLK - 1),
                )
            if dirn == "b":
                obs = core.tile([128, T], F16, tag="obs", name="obs")
                nc.scalar.activation(out=obs[:], in_=po[:], func=ACT.Copy)
                dma(ob[mt][:, t0:t0 + T], obs[:])
            else:
                cb_rev = nchunk - 1 - c
                obs = core.tile([128, T], F16, tag="obs", name="obs")
                dma(obs, ob[mt][:, cb_rev * T:(cb_rev + 1) * T])
                oo = core.tile([128, T], F32, tag="oo", name="oo")
                nc.vector.tensor_add(oo[:], po[:], obs[:, ::-1])
                dma(y_param[mt * 128:(mt + 1) * 128, t0:t0 + T], oo[:])


def build_nc(L, T):
    nc = bacc.Bacc("TRN2", target_bir_lowering=False, debug=False)
    prm = {}
    prm["u_f"] = nc.declare_dram_parameter("u_f", [D_MODEL, L + KCONV - 1], F16, isOutput=False)
    prm["u_b"] = nc.declare_dram_parameter("u_b", [D_MODEL, L + KCONV - 1], F16, isOutput=False)
    for d in ("f", "b"):
        prm[f"{d}_wx"] = nc.declare_dram_parameter(f"{d}_wx", [2 * KCONV, 128, D_INNER], F16, isOutput=False)
        prm[f"{d}_wz"] = nc.declare_dram_parameter(f"{d}_wz", [2, 128, D_INNER], F16, isOutput=False)
        prm[f"{d}_wxp"] = nc.declare_dram_parameter(f"{d}_wxp", [NBLK, 128, DT_RANK + 2 * NSTATE], F16, isOutput=False)
        prm[f"{d}_wdt"] = nc.declare_dram_parameter(f"{d}_wdt", [DT_RANK, D_INNER], F16, isOutput=False)
        prm[f"{d}_wo"] = nc.declare_dram_parameter(f"{d}_wo", [NBLK, 128, D_MODEL], F16, isOutput=False)
        prm[f"{d}_cb"] = nc.declare_dram_parameter(f"{d}_cb", [NBLK, 128, 1], F32, isOutput=False)
        prm[f"{d}_dtb"] = nc.declare_dram_parameter(f"{d}_dtb", [NBLK, 128, 1], F32, isOutput=False)
        prm[f"{d}_A"] = nc.declare_dram_parameter(f"{d}_A", [NBLK, 128, NSTATE], F32, isOutput=False)
        prm[f"{d}_D"] = nc.declare_dram_parameter(f"{d}_D", [NBLK, 128, 1], F32, isOutput=False)
    y_param = nc.declare_dram_parameter("y", [D_MODEL, L], F32, isOutput=True)

    with contextlib.ExitStack() as ctx:
        tc = ctx.enter_context(tile.TileContext(nc))
        pools = {
            "weights": ctx.enter_context(tc.tile_pool(name="weights", bufs=1)),
            "fullL": ctx.enter_context(tc.tile_pool(name="fullL", bufs=1)),
            "core": ctx.enter_context(tc.tile_pool(name="core", bufs=2)),
            "psum": ctx.enter_context(tc.tile_pool(name="psum", bufs=1, space="PSUM")),
            "dram": ctx.enter_context(tc.tile_pool(name="dram", bufs=1, space="DRAM")),
            "small": ctx.enter_context(tc.tile_pool(name="small", bufs=1)),
        }
        ob = [pools["dram"].tile([128, L], F16, tag=f"ob{m}", name=f"ob{m}") for m in range(NMT)]
        for dirn in ("b", "f"):
            _emit_dir(nc, pools, prm, dirn, L, T, ob, y_param[:])
    nc.compile()
    return nc


@functools.lru_cache(maxsize=None)
def _get_nc(L, T):
    return build_nc(L, T)


def _prep_dir_weights(pfx, inputs):
    in_w = np.asarray(inputs[pfx + "_in_w"], np.float32)      # [1024, 256]
    conv_w = np.asarray(inputs[pfx + "_conv_w"], np.float32)  # [512, 4]
    conv_b = np.asarray(inputs[pfx + "_conv_b"], np.float32)  # [512]
    xproj_w = np.asarray(inputs[pfx + "_xproj_w"], np.float32)  # [48, 512]
    dt_w = np.asarray(inputs[pfx + "_dt_w"], np.float32)      # [512, 16]
    dt_b = np.asarray(inputs[pfx + "_dt_b"], np.float32)      # [512]
    A_log = np.asarray(inputs[pfx + "_A_log"], np.float32)    # [512, 16]
    Dp = np.asarray(inputs[pfx + "_D"], np.float32)           # [512]
    out_w = np.asarray(inputs[pfx + "_out_w"], np.float32)    # [256, 512]

    in_w_x = in_w[:D_INNER]        # [512, 256]
    in_w_z = in_w[D_INNER:]        # [512, 256]
    wx = np.zeros([2 * KCONV, 128, D_INNER], np.float16)
    for k in range(KCONV):
        for dmb in range(2):
            # lhsT[(k,dmb) block][r, d] = in_w_x[d, dmb*128+r] * conv_w[d, k]
            wx[k * 2 + dmb] = (
                in_w_x[:, dmb * 128:(dmb + 1) * 128] * conv_w[:, k:k + 1]
            ).T.astype(np.float16)
    wz = np.stack([
        in_w_z[:, dmb * 128:(dmb + 1) * 128].T for dmb in range(2)
    ]).astype(np.float16)          # [2, 128, 512]
    wxp = np.stack([
        xproj_w[:, b * 128:(b + 1) * 128].T for b in range(NBLK)
    ]).astype(np.float16)          # [4, 128, 48]
    wdt = dt_w.T.astype(np.float16)  # [16, 512]
    wo = np.stack([
        0.5 * out_w[:, b * 128:(b + 1) * 128].T for b in range(NBLK)
    ]).astype(np.float16)          # [4, 128, 256]
    A = (-np.exp(A_log)).astype(np.float32)
    return {
        f"{pfx}_wx": wx,
        f"{pfx}_wz": wz,
        f"{pfx}_wxp": wxp,
        f"{pfx}_wdt": wdt,
        f"{pfx}_wo": wo,
        f"{pfx}_cb": conv_b.reshape(NBLK, 128, 1).astype(np.float32),
        f"{pfx}_dtb": dt_b.reshape(NBLK, 128, 1).astype(np.float32),
        f"{pfx}_A": A.reshape(NBLK, 128, NSTATE),
        f"{pfx}_D": Dp.reshape(NBLK, 128, 1).astype(np.float32),
    }


def make_in_maps(inputs, L):
    hs = np.asarray(inputs["hidden_states"], np.float32)  # [B, L, 256]
    B = hs.shape[0]
    wmap = {}
    for pfx in ("f", "b"):
        wmap.update(_prep_dir_weights(pfx, inputs))
    in_maps = []
    for c in range(B):
        u = np.ascontiguousarray(hs[c].T)  # [256, L]
        pad = np.zeros([D_MODEL, KCONV - 1], np.float32)
        u_f = np.concatenate([pad, u], axis=1).astype(np.float16)
        u_b = np.concatenate([pad, u[:, ::-1]], axis=1).astype(np.float16)
        m = dict(wmap)
        m["u_f"] = u_f
        m["u_b"] = u_b
        in_maps.append(m)
    return in_maps


def run(inputs, trace=False, **kwargs):
    from concourse.bass_utils import run_bass_kernel_spmd

    hs = np.asarray(inputs["hidden_states"], np.float32)
    B, L, _ = hs.shape
    nc = _get_nc(L, T_FULL if L % T_FULL == 0 else L)
    in_maps = make_in_maps(inputs, L)
    res = run_bass_kernel_spmd(nc, in_maps, list(range(N_CORES))[:B],
                               trace=trace, **kwargs)
    out = np.stack([
        np.asarray(res.results[c]["y"]).T for c in range(B)
    ]).astype(np.float32)
    return out, res


def kernel(**inputs):
    return run(inputs)[0]


# revision 14
# speedup vs baseline: 1.5323x; 1.5323x over previous
"""Bidirectional Mamba block on 8 Trainium2 NeuronCores.

Strategy
--------
Data-parallel over batch: each of the 8 cores runs one batch element
(both directions) end to end; no collectives.

Per core / per direction, with d_inner=512 split into 4 blocks of 128
partitions and the state dim n=16 split into 4 groups of 4:

  phase 1: x_conv = silu(conv1d(u @ in_w_x.T) + conv_b)  -- the depthwise
           causal conv is folded into the input projection on the PE by
           expanding the contraction dim to (4 taps x 256 d_model) with
           host-combined weights;  sz = silu(u @ in_w_z.T).
  phase 2: dbl = x_conv @ xproj.T (PE), delta = softplus(dt-proj + dt_b)
           (PE + one ACT Softplus pass). B/C rows of dbl bounce through a
           DRAM scratch so a stride-0-partition DMA can broadcast them to
           all 128 partitions.
  phase 3: per (block, n-group): a = exp(A[:,n] * delta) on ACT (scale is
           a per-partition AP, so the multiply is free), b = w * B_bcast,
           h = tensor_tensor_scan(a, b) (the native DVE linear-recurrence
           instruction, chained across t-chunks via per-partition initial
           values), y = sum_n h * C_bcast via a small add tree, then
           y = (y + x_conv * D) * sz, and the output projection on the PE
           (0.5 factor folded into out_w host-side).

The backward direction runs first on a host-reversed copy of the input;
its output (y_T) is stored to DRAM and combined, reversed, with the
forward direction's output projection.
"""

import contextlib
import functools
import sys

for _p in ("/opt/trn_rl_repo",):
    if _p not in sys.path:
        sys.path.insert(0, _p)

import numpy as np

import concourse.bass as bass
import concourse.bacc as bacc
import concourse.mybir as mybir
import concourse.tile as tile

F16 = mybir.dt.float16
F32 = mybir.dt.float32
AOP = mybir.AluOpType
ACT = mybir.ActivationFunctionType

D_MODEL = 256
D_INNER = 512
NSTATE = 16
DT_RANK = 16
KCONV = 4
NBLK = D_INNER // 128   # 4 d_inner blocks
NMT = D_MODEL // 128    # 2 d_model tiles
NJ = 4                  # n per group
NG = NSTATE // NJ       # 4 groups
N_CORES = 8

L_FULL = 4096
T_FULL = 512

# Native Silu ACT function exists on hardware but is not implemented by
# CoreSim; sim_test flips this to False to use sigmoid+multiply instead.
SILU_NATIVE = True
SOFTPLUS_NATIVE = False  # no ACT table provides Softplus on this toolchain


def _patch_act_tables():
    """Keep Exp and Ln in one table set (natural_log_exp_and_others) so the
    softplus (Exp+Ln) and the a-gen Exps never force ACT table reloads.
    Entries are blanked in place (positions preserved) because the emitted
    act_func_set_id indexes act_info.json by position."""
    import concourse.bacc as _bacc
    import concourse.hw_specs as _hw

    if getattr(_bacc, "_mamba_act_patch", False):
        return
    real = _hw.get_activation_tables

    def patched(arch):
        tabs = dict(real(arch))
        for nm in ("exp_and_others", "exp_and_friends", "natural_log"):
            if nm in tabs:
                tabs[nm] = set()
        return tabs

    _bacc.get_activation_tables = patched
    _bacc._mamba_act_patch = True


def _emit_dir(nc, pools, prm, dirn, L, T, ob, y_param):
    nchunk = L // T
    wp, fl, hp, core, ps, dram, sm = (
        pools["weights"], pools["fullL"], pools["half"], pools["core"],
        pools["psum"], pools["dram"], pools["small"],
    )

    def dma(out, in_):
        nc.sync.dma_start(out=out, in_=in_)

    # ---- weights into SBUF ----
    wx = []
    for i in range(2 * KCONV):
        t = wp.tile([128, D_INNER], F16, tag=f"wx{i}", name=f"wx{i}")
        dma(t, prm[f"{dirn}_wx"][i])
        wx.append(t)
    wz = []
    for i in range(2):
        t = wp.tile([128, D_INNER], F16, tag=f"wz{i}", name=f"wz{i}")
        dma(t, prm[f"{dirn}_wz"][i])
        wz.append(t)
    wxp = []
    for b in range(NBLK):
        t = wp.tile([128, DT_RANK + 2 * NSTATE], F16, tag=f"wxp{b}", name=f"wxp{b}")
        dma(t, prm[f"{dirn}_wxp"][b])
        wxp.append(t)
    wdt = wp.tile([DT_RANK, D_INNER], F16, tag="wdt", name="wdt")
    dma(wdt, prm[f"{dirn}_wdt"][:])
    wo = []
    for b in range(NBLK):
        t = wp.tile([128, D_MODEL], F16, tag=f"wo{b}", name=f"wo{b}")
        dma(t, prm[f"{dirn}_wo"][b])
        wo.append(t)
    cb, dtb, At, Dd = [], [], [], []
    for b in range(NBLK):
        t = wp.tile([128, 1], F32, tag=f"cb{b}", name=f"cb{b}")
        dma(t, prm[f"{dirn}_cb"][b])
        cb.append(t)
        t = wp.tile([128, 1], F32, tag=f"dtb{b}", name=f"dtb{b}")
        dma(t, prm[f"{dirn}_dtb"][b])
        dtb.append(t)
        t = wp.tile([128, NSTATE], F32, tag=f"A{b}", name=f"A{b}")
        dma(t, prm[f"{dirn}_A"][b])
        At.append(t)
        t = wp.tile([128, 1], F32, tag=f"D{b}", name=f"D{b}")
        dma(t, prm[f"{dirn}_D"][b])
        Dd.append(t)

    u_param = prm[f"u_{dirn}"]

    # ---- persistent per-direction tensors ----
    xc = [fl.tile([128, L], F16, tag=f"xc{b}", name=f"xc{b}") for b in range(NBLK)]
    dl = [fl.tile([128, L], F16, tag=f"dl{b}", name=f"dl{b}") for b in range(NBLK)]
    szd = [dram.tile([128, L], F16, tag=f"szd{b}", name=f"szd{b}") for b in range(NBLK)]
    state = [sm.tile([128, NSTATE], F32, tag=f"st{b}", name=f"st{b}") for b in range(NBLK)]
    bc = dram.tile([DT_RANK + 2 * NSTATE - DT_RANK, L], F16, tag="bc", name="bc")  # [32, L]

    # ---- phase 1: x_conv (conv folded into in-proj) and sz ----
    for c in range(nchunk):
        t0 = c * T
        u0 = core.tile([128, T + KCONV - 1], F16, tag="u0", name="u0")
        u1 = core.tile([128, T + KCONV - 1], F16, tag="u1", name="u1")
        dma(u0, u_param[0:128, t0:t0 + T + KCONV - 1])
        dma(u1, u_param[128:256, t0:t0 + T + KCONV - 1])
        uu = (u0, u1)
        for b in range(NBLK):
            px = ps.tile([128, T], F32, tag="px", name="px", bufs=2)
            for kb in range(2 * KCONV):
                k, dmb = divmod(kb, 2)
                nc.tensor.matmul(
                    px[:], wx[kb][:, b * 128:(b + 1) * 128],
                    uu[dmb][:, k:k + T],
                    start=(kb == 0), stop=(kb == 2 * KCONV - 1),
                )
            if SILU_NATIVE:
                nc.scalar.activation(
                    out=xc[b][:, t0:t0 + T], in_=px[:], func=ACT.Silu,
                    bias=cb[b][:], scale=1.0,
                )
            else:
                xq = core.tile([128, T], F32, tag="xq", name="xq")
                nc.vector.tensor_scalar(out=xq[:], in0=px[:], scalar1=cb[b][:],
                                        scalar2=None, op0=AOP.add)
                sg = core.tile([128, T], F32, tag="sg", name="sg")
                nc.scalar.activation(out=sg[:], in_=xq[:], func=ACT.Sigmoid)
                nc.vector.tensor_mul(xc[b][:, t0:t0 + T], xq[:], sg[:])
        for b in range(NBLK):
            pz = ps.tile([128, T], F32, tag="pz", name="pz")
            for dmb in range(2):
                nc.tensor.matmul(
                    pz[:], wz[dmb][:, b * 128:(b + 1) * 128],
                    uu[dmb][:, KCONV - 1:KCONV - 1 + T],
                    start=(dmb == 0), stop=(dmb == 1),
                )
            szc = core.tile([128, T], F16, tag="szc", name="szc")
            if SILU_NATIVE:
                nc.scalar.activation(out=szc[:], in_=pz[:], func=ACT.Silu)
            else:
                sg2 = core.tile([128, T], F32, tag="sg2", name="sg2")
                nc.scalar.activation(out=sg2[:], in_=pz[:], func=ACT.Sigmoid)
                nc.vector.tensor_mul(szc[:], pz[:], sg2[:])
            dma(szd[b][:, t0:t0 + T], szc[:])

    # ---- phase 2: dbl (dt/B/C) and delta ----
    for c in range(nchunk):
        t0 = c * T
        pd = ps.tile([DT_RANK + 2 * NSTATE, T], F32, tag="pd", name="pd")
        for b in range(NBLK):
            nc.tensor.matmul(
                pd[:], wxp[b][:], xc[b][:, t0:t0 + T],
                start=(b == 0), stop=(b == NBLK - 1),
            )
        dbl = core.tile([DT_RANK + 2 * NSTATE, T], F16, tag="dbl", name="dbl")
        nc.scalar.activation(out=dbl[:], in_=pd[:], func=ACT.Copy)
        dma(bc[:, t0:t0 + T], dbl[DT_RANK:, :])
        for b in range(NBLK):
            pt = ps.tile([128, T], F32, tag="pt", name="pt")
            nc.tensor.matmul(
                pt[:], wdt[:, b * 128:(b + 1) * 128], dbl[0:DT_RANK, :],
                start=True, stop=True,
            )
            if SOFTPLUS_NATIVE:
                nc.scalar.activation(out=dl[b][:, t0:t0 + T], in_=pt[:],
                                     func=ACT.Softplus, bias=dtb[b][:], scale=1.0)
            else:
                et = core.tile([128, T], F32, tag="et", name="et")
                nc.scalar.activation(out=et[:], in_=pt[:], func=ACT.Exp,
                                     bias=dtb[b][:], scale=1.0)
                nc.scalar.activation(out=dl[b][:, t0:t0 + T], in_=et[:],
                                     func=ACT.Ln, bias=1.0, scale=1.0)

    # ---- phase 3: selective scan + gating + out-proj ----
    # L is processed in halves; within a half, each n-group's B/C rows are
    # broadcast once (via the DRAM bounce, partition-stride-0 read) and all
    # blocks/chunks consume them; per-group partial y sums accumulate into
    # yacc (g == 0 writes, g > 0 adds).
    NHL = 2 if L >= 2 * T else 1
    HL = L // NHL
    NCH = HL // T
    bc_ap = bc[:]
    for lh in range(NHL):
        h0 = lh * HL
        wh = []
        for b in range(NBLK):
            t = hp.tile([128, HL], F16, tag=f"wh{b}", name=f"wh{b}")
            nc.vector.tensor_mul(t[:], dl[b][:, h0:h0 + HL], xc[b][:, h0:h0 + HL])
            wh.append(t)
        yacc = [hp.tile([128, HL], F16, tag=f"ya{b}", name=f"ya{b}")
                for b in range(NBLK)]
        for g in range(NG):
            Bb = hp.tile([128, NJ, HL], F16, tag="Bb", name="Bb")
            dma(Bb, bass.AP(
                tensor=bc_ap.tensor,
                offset=bc_ap.offset + (g * NJ) * L + h0,
                ap=[[0, 128], [L, NJ], [1, HL]],
            ))
            Cb = hp.tile([128, NJ, HL], F16, tag="Cb", name="Cb")
            dma(Cb, bass.AP(
                tensor=bc_ap.tensor,
                offset=bc_ap.offset + (NSTATE + g * NJ) * L + h0,
                ap=[[0, 128], [L, NJ], [1, HL]],
            ))
            for ci in range(NCH):
                c = lh * NCH + ci
                s0 = ci * T
                for b in range(NBLK):
                    wh_ap = wh[b][:]
                    wrep = bass.AP(
                        tensor=wh_ap.tensor, offset=wh_ap.offset + s0,
                        ap=[wh_ap.ap[0], [0, NJ], [1, T]],
                    )
                    at = core.tile([128, NJ, T], F16, tag="at", name="at")
                    for j in range(NJ):
                        n = g * NJ + j
                        nc.scalar.activation(
                            out=at[:, j, :], in_=dl[b][:, h0 + s0:h0 + s0 + T],
                            func=ACT.Exp, scale=At[b][:, n:n + 1],
                        )
                    bt = core.tile([128, NJ, T], F16, tag="bt", name="bt", bufs=1)
                    nc.vector.tensor_mul(bt[:], wrep, Bb[:, :, s0:s0 + T])
                    ht = core.tile([128, NJ, T], F16, tag="ht", name="ht")
                    for j in range(NJ):
                        n = g * NJ + j
                        init = 0.0 if c == 0 else state[b][:, n:n + 1]
                        nc.vector.tensor_tensor_scan(
                            ht[:, j, :], at[:, j, :], bt[:, j, :], init,
                            AOP.mult, AOP.add,
                        )
                    nc.vector.tensor_copy(
                        out=state[b][:, g * NJ:(g + 1) * NJ], in_=ht[:, :, T - 1],
                    )
                    pt2 = core.tile([128, NJ, T], F16, tag="pt2", name="pt2", bufs=1)
                    nc.vector.tensor_mul(pt2[:], ht[:], Cb[:, :, s0:s0 + T])
                    q1 = core.tile([128, 2, T], F16, tag="q1", name="q1")
                    nc.vector.tensor_add(q1[:], pt2[:, 0:NJ:2, :], pt2[:, 1:NJ:2, :])
                    if g == 0:
                        nc.vector.tensor_add(
                            yacc[b][:, s0:s0 + T], q1[:, 0, :], q1[:, 1, :])
                    else:
                        qr = core.tile([128, T], F16, tag="qr", name="qr")
                        nc.vector.tensor_add(qr[:], q1[:, 0, :], q1[:, 1, :])
                        nc.vector.tensor_add(
                            yacc[b][:, s0:s0 + T], yacc[b][:, s0:s0 + T], qr[:])
        # gating + out-proj for this half
        for ci in range(NCH):
            c = lh * NCH + ci
            t0 = c * T
            s0 = ci * T
            gts = []
            for b in range(NBLK):
                szc2 = core.tile([128, T], F16, tag="szc2", name="szc2")
                dma(szc2, szd[b][:, t0:t0 + T])
                y1 = core.tile([128, T], F16, tag="y1", name="y1")
                nc.vector.scalar_tensor_tensor(
                    out=y1[:], in0=xc[b][:, t0:t0 + T], scalar=Dd[b][:],
                    in1=yacc[b][:, s0:s0 + T], op0=AOP.mult, op1=AOP.add,
                )
                gt = core.tile([128, T], F16, tag=f"gt{b}", name=f"gt{b}")
                nc.vector.tensor_mul(gt[:], y1[:], szc2[:])
                gts.append(gt)
            for mt in range(NMT):
                po = ps.tile([128, T], F32, tag="po", name="po", bufs=2)
                for b in range(NBLK):
                    nc.tensor.matmul(
                        po[:], wo[b][:, mt * 128:(mt + 1) * 128], gts[b][:],
                        start=(b == 0), stop=(b == NBLK - 1),
                    )
                if dirn == "b":
                    obs = core.tile([128, T], F16, tag="obs", name="obs")
                    nc.scalar.activation(out=obs[:], in_=po[:], func=ACT.Copy)
                    dma(ob[mt][:, t0:t0 + T], obs[:])
                else:
                    cb_rev = (L // T) - 1 - c
                    obs = core.tile([128, T], F16, tag="obs", name="obs")
                    dma(obs, ob[mt][:, cb_rev * T:(cb_rev + 1) * T])
                    oo = core.tile([128, T], F32, tag="oo", name="oo")
                    nc.vector.tensor_add(oo[:], po[:], obs[:, ::-1])
                    dma(y_param[mt * 128:(mt + 1) * 128, t0:t0 + T], oo[:])


def build_nc(L, T):
    _patch_act_tables()
    nc = bacc.Bacc("TRN2", target_bir_lowering=False, debug=False)
    prm = {}
    prm["u_f"] = nc.declare_dram_parameter("u_f", [D_MODEL, L + KCONV - 1], F16, isOutput=False)
    prm["u_b"] = nc.declare_dram_parameter("u_b", [D_MODEL, L + KCONV - 1], F16, isOutput=False)
    for d in ("f", "b"):
        prm[f"{d}_wx"] = nc.declare_dram_parameter(f"{d}_wx", [2 * KCONV, 128, D_INNER], F16, isOutput=False)
        prm[f"{d}_wz"] = nc.declare_dram_parameter(f"{d}_wz", [2, 128, D_INNER], F16, isOutput=False)
        prm[f"{d}_wxp"] = nc.declare_dram_parameter(f"{d}_wxp", [NBLK, 128, DT_RANK + 2 * NSTATE], F16, isOutput=False)
        prm[f"{d}_wdt"] = nc.declare_dram_parameter(f"{d}_wdt", [DT_RANK, D_INNER], F16, isOutput=False)
        prm[f"{d}_wo"] = nc.declare_dram_parameter(f"{d}_wo", [NBLK, 128, D_MODEL], F16, isOutput=False)
        prm[f"{d}_cb"] = nc.declare_dram_parameter(f"{d}_cb", [NBLK, 128, 1], F32, isOutput=False)
        prm[f"{d}_dtb"] = nc.declare_dram_parameter(f"{d}_dtb", [NBLK, 128, 1], F32, isOutput=False)
        prm[f"{d}_A"] = nc.declare_dram_parameter(f"{d}_A", [NBLK, 128, NSTATE], F32, isOutput=False)
        prm[f"{d}_D"] = nc.declare_dram_parameter(f"{d}_D", [NBLK, 128, 1], F32, isOutput=False)
    y_param = nc.declare_dram_parameter("y", [D_MODEL, L], F32, isOutput=True)

    with contextlib.ExitStack() as ctx:
        tc = ctx.enter_context(tile.TileContext(nc))
        pools = {
            "weights": ctx.enter_context(tc.tile_pool(name="weights", bufs=1)),
            "fullL": ctx.enter_context(tc.tile_pool(name="fullL", bufs=1)),
            "half": ctx.enter_context(tc.tile_pool(name="half", bufs=1)),
            "core": ctx.enter_context(tc.tile_pool(name="core", bufs=2)),
            "psum": ctx.enter_context(tc.tile_pool(name="psum", bufs=1, space="PSUM")),
            "dram": ctx.enter_context(tc.tile_pool(name="dram", bufs=1, space="DRAM")),
            "small": ctx.enter_context(tc.tile_pool(name="small", bufs=1)),
        }
        ob = [pools["dram"].tile([128, L], F16, tag=f"ob{m}", name=f"ob{m}") for m in range(NMT)]
        for dirn in ("b", "f"):
            _emit_dir(nc, pools, prm, dirn, L, T, ob, y_param[:])
    nc.compile()
    return nc


@functools.lru_cache(maxsize=None)
def _get_nc(L, T):
    return build_nc(L, T)


def _prep_dir_weights(pfx, inputs):
    in_w = np.asarray(inputs[pfx + "_in_w"], np.float32)      # [1024, 256]
    conv_w = np.asarray(inputs[pfx + "_conv_w"], np.float32)  # [512, 4]
    conv_b = np.asarray(inputs[pfx + "_conv_b"], np.float32)  # [512]
    xproj_w = np.asarray(inputs[pfx + "_xproj_w"], np.float32)  # [48, 512]
    dt_w = np.asarray(inputs[pfx + "_dt_w"], np.float32)      # [512, 16]
    dt_b = np.asarray(inputs[pfx + "_dt_b"], np.float32)      # [512]
    A_log = np.asarray(inputs[pfx + "_A_log"], np.float32)    # [512, 16]
    Dp = np.asarray(inputs[pfx + "_D"], np.float32)           # [512]
    out_w = np.asarray(inputs[pfx + "_out_w"], np.float32)    # [256, 512]

    in_w_x = in_w[:D_INNER]        # [512, 256]
    in_w_z = in_w[D_INNER:]        # [512, 256]
    wx = np.zeros([2 * KCONV, 128, D_INNER], np.float16)
    for k in range(KCONV):
        for dmb in range(2):
            # lhsT[(k,dmb) block][r, d] = in_w_x[d, dmb*128+r] * conv_w[d, k]
            wx[k * 2 + dmb] = (
                in_w_x[:, dmb * 128:(dmb + 1) * 128] * conv_w[:, k:k + 1]
            ).T.astype(np.float16)
    wz = np.stack([
        in_w_z[:, dmb * 128:(dmb + 1) * 128].T for dmb in range(2)
    ]).astype(np.float16)          # [2, 128, 512]
    wxp = np.stack([
        xproj_w[:, b * 128:(b + 1) * 128].T for b in range(NBLK)
    ]).astype(np.float16)          # [4, 128, 48]
    wdt = dt_w.T.astype(np.float16)  # [16, 512]
    wo = np.stack([
        0.5 * out_w[:, b * 128:(b + 1) * 128].T for b in range(NBLK)
    ]).astype(np.float16)          # [4, 128, 256]
    A = (-np.exp(A_log)).astype(np.float32)
    return {
        f"{pfx}_wx": wx,
        f"{pfx}_wz": wz,
        f"{pfx}_wxp": wxp,
        f"{pfx}_wdt": wdt,
        f"{pfx}_wo": wo,
        f"{pfx}_cb": conv_b.reshape(NBLK, 128, 1).astype(np.float32),
        f"{pfx}_dtb": dt_b.reshape(NBLK, 128, 1).astype(np.float32),
        f"{pfx}_A": A.reshape(NBLK, 128, NSTATE),
        f"{pfx}_D": Dp.reshape(NBLK, 128, 1).astype(np.float32),
    }


def make_in_maps(inputs, L):
    hs = np.asarray(inputs["hidden_states"], np.float32)  # [B, L, 256]
    B = hs.shape[0]
    wmap = {}
    for pfx in ("f", "b"):
        wmap.update(_prep_dir_weights(pfx, inputs))
    in_maps = []
    for c in range(B):
        u = np.ascontiguousarray(hs[c].T)  # [256, L]
        pad = np.zeros([D_MODEL, KCONV - 1], np.float32)
        u_f = np.concatenate([pad, u], axis=1).astype(np.float16)
        u_b = np.concatenate([pad, u[:, ::-1]], axis=1).astype(np.float16)
        m = dict(wmap)
        m["u_f"] = u_f
        m["u_b"] = u_b
        in_maps.append(m)
    return in_maps


def run(inputs, trace=False, **kwargs):
    from concourse.bass_utils import run_bass_kernel_spmd

    hs = np.asarray(inputs["hidden_states"], np.float32)
    B, L, _ = hs.shape
    nc = _get_nc(L, T_FULL if L % T_FULL == 0 else L)
    in_maps = make_in_maps(inputs, L)
    res = run_bass_kernel_spmd(nc, in_maps, list(range(N_CORES))[:B],
                               trace=trace, **kwargs)
    out = np.stack([
        np.asarray(res.results[c]["y"]).T for c in range(B)
    ]).astype(np.float32)
    return out, res


def kernel(**inputs):
    return run(inputs)[0]


# revision 16
# speedup vs baseline: 1.6476x; 1.0753x over previous
"""Bidirectional Mamba block on 8 Trainium2 NeuronCores.

Strategy
--------
Data-parallel over batch: each of the 8 cores runs one batch element
(both directions) end to end; no collectives.

Per core / per direction, with d_inner=512 split into 4 blocks of 128
partitions and the state dim n=16 split into 4 groups of 4:

  phase 1: x_conv = silu(conv1d(u @ in_w_x.T) + conv_b)  -- the depthwise
           causal conv is folded into the input projection on the PE by
           expanding the contraction dim to (4 taps x 256 d_model) with
           host-combined weights;  sz = silu(u @ in_w_z.T).
  phase 2: dbl = x_conv @ xproj.T (PE), delta = softplus(dt-proj + dt_b)
           (PE + one ACT Softplus pass). B/C rows of dbl bounce through a
           DRAM scratch so a stride-0-partition DMA can broadcast them to
           all 128 partitions.
  phase 3: per (block, n-group): a = exp(A[:,n] * delta) on ACT (scale is
           a per-partition AP, so the multiply is free), b = w * B_bcast,
           h = tensor_tensor_scan(a, b) (the native DVE linear-recurrence
           instruction, chained across t-chunks via per-partition initial
           values), y = sum_n h * C_bcast via a small add tree, then
           y = (y + x_conv * D) * sz, and the output projection on the PE
           (0.5 factor folded into out_w host-side).

The backward direction runs first on a host-reversed copy of the input;
its output (y_T) is stored to DRAM and combined, reversed, with the
forward direction's output projection.
"""

import contextlib
import functools
import sys

for _p in ("/opt/trn_rl_repo",):
    if _p not in sys.path:
        sys.path.insert(0, _p)

import numpy as np

import concourse.bass as bass
import concourse.bacc as bacc
import concourse.mybir as mybir
import concourse.tile as tile

F16 = mybir.dt.float16
F32 = mybir.dt.float32
AOP = mybir.AluOpType
ACT = mybir.ActivationFunctionType

D_MODEL = 256
D_INNER = 512
NSTATE = 16
DT_RANK = 16
KCONV = 4
NBLK = D_INNER // 128   # 4 d_inner blocks
NMT = D_MODEL // 128    # 2 d_model tiles
NJ = 4                  # n per group
NG = NSTATE // NJ       # 4 groups
N_CORES = 8

L_FULL = 4096
T_FULL = 512

# Native Silu ACT function exists on hardware but is not implemented by
# CoreSim; sim_test flips this to False to use sigmoid+multiply instead.
SILU_NATIVE = True
SOFTPLUS_NATIVE = False  # no ACT table provides Softplus on this toolchain


def _patch_act_tables():
    """Keep Exp and Ln in one table set (natural_log_exp_and_others) so the
    softplus (Exp+Ln) and the a-gen Exps never force ACT table reloads.
    Entries are blanked in place (positions preserved) because the emitted
    act_func_set_id indexes act_info.json by position."""
    import concourse.bacc as _bacc
    import concourse.hw_specs as _hw

    if getattr(_bacc, "_mamba_act_patch", False):
        return
    real = _hw.get_activation_tables

    def patched(arch):
        tabs = dict(real(arch))
        for nm in ("exp_and_others", "exp_and_friends", "natural_log"):
            if nm in tabs:
                tabs[nm] = set()
        return tabs

    _bacc.get_activation_tables = patched
    _bacc._mamba_act_patch = True


def _emit_dir(nc, pools, prm, dirn, L, T, ob, y_param):
    nchunk = L // T
    wp, fl, hp, core, ps, dram, sm = (
        pools["weights"], pools["fullL"], pools["half"], pools["core"],
        pools["psum"], pools["dram"], pools["small"],
    )

    def dma(out, in_):
        nc.sync.dma_start(out=out, in_=in_)

    # ---- weights into SBUF ----
    wx = []
    for i in range(2 * KCONV):
        t = wp.tile([128, D_INNER], F16, tag=f"wx{i}", name=f"wx{i}")
        dma(t, prm[f"{dirn}_wx"][i])
        wx.append(t)
    wz = []
    for i in range(2):
        t = wp.tile([128, D_INNER], F16, tag=f"wz{i}", name=f"wz{i}")
        dma(t, prm[f"{dirn}_wz"][i])
        wz.append(t)
    wxp = []
    for b in range(NBLK):
        t = wp.tile([128, DT_RANK + 2 * NSTATE], F16, tag=f"wxp{b}", name=f"wxp{b}")
        dma(t, prm[f"{dirn}_wxp"][b])
        wxp.append(t)
    wdt = wp.tile([DT_RANK, D_INNER], F16, tag="wdt", name="wdt")
    dma(wdt, prm[f"{dirn}_wdt"][:])
    wo = []
    for b in range(NBLK):
        t = wp.tile([128, D_MODEL], F16, tag=f"wo{b}", name=f"wo{b}")
        dma(t, prm[f"{dirn}_wo"][b])
        wo.append(t)
    cb, dtb, At, Dd = [], [], [], []
    for b in range(NBLK):
        t = wp.tile([128, 1], F32, tag=f"cb{b}", name=f"cb{b}")
        dma(t, prm[f"{dirn}_cb"][b])
        cb.append(t)
        t = wp.tile([128, 1], F32, tag=f"dtb{b}", name=f"dtb{b}")
        dma(t, prm[f"{dirn}_dtb"][b])
        dtb.append(t)
        t = wp.tile([128, NSTATE], F32, tag=f"A{b}", name=f"A{b}")
        dma(t, prm[f"{dirn}_A"][b])
        At.append(t)
        t = wp.tile([128, 1], F32, tag=f"D{b}", name=f"D{b}")
        dma(t, prm[f"{dirn}_D"][b])
        Dd.append(t)

    u_param = prm[f"u_{dirn}"]

    # ---- persistent per-direction tensors ----
    xc = [fl.tile([128, L], F16, tag=f"xc{b}", name=f"xc{b}") for b in range(NBLK)]
    dl = [fl.tile([128, L], F16, tag=f"dl{b}", name=f"dl{b}") for b in range(NBLK)]
    szd = [dram.tile([128, L], F16, tag=f"szd{b}", name=f"szd{b}") for b in range(NBLK)]
    state = [sm.tile([128, NSTATE], F32, tag=f"st{b}", name=f"st{b}") for b in range(NBLK)]
    bc = dram.tile([DT_RANK + 2 * NSTATE - DT_RANK, L], F16, tag="bc", name="bc")  # [32, L]

    # ---- phase 1: x_conv (conv folded into in-proj) and sz ----
    for c in range(nchunk):
        t0 = c * T
        u0 = core.tile([128, T + KCONV - 1], F16, tag="u0", name="u0")
        u1 = core.tile([128, T + KCONV - 1], F16, tag="u1", name="u1")
        dma(u0, u_param[0:128, t0:t0 + T + KCONV - 1])
        dma(u1, u_param[128:256, t0:t0 + T + KCONV - 1])
        uu = (u0, u1)
        for b in range(NBLK):
            px = ps.tile([128, T], F32, tag="px", name="px", bufs=2)
            for kb in range(2 * KCONV):
                k, dmb = divmod(kb, 2)
                nc.tensor.matmul(
                    px[:], wx[kb][:, b * 128:(b + 1) * 128],
                    uu[dmb][:, k:k + T],
                    start=(kb == 0), stop=(kb == 2 * KCONV - 1),
                )
            if SILU_NATIVE:
                nc.scalar.activation(
                    out=xc[b][:, t0:t0 + T], in_=px[:], func=ACT.Silu,
                    bias=cb[b][:], scale=1.0,
                )
            else:
                xq = core.tile([128, T], F32, tag="xq", name="xq")
                nc.vector.tensor_scalar(out=xq[:], in0=px[:], scalar1=cb[b][:],
                                        scalar2=None, op0=AOP.add)
                sg = core.tile([128, T], F32, tag="sg", name="sg")
                nc.scalar.activation(out=sg[:], in_=xq[:], func=ACT.Sigmoid)
                nc.vector.tensor_mul(xc[b][:, t0:t0 + T], xq[:], sg[:])
        for b in range(NBLK):
            pz = ps.tile([128, T], F32, tag="pz", name="pz")
            for dmb in range(2):
                nc.tensor.matmul(
                    pz[:], wz[dmb][:, b * 128:(b + 1) * 128],
                    uu[dmb][:, KCONV - 1:KCONV - 1 + T],
                    start=(dmb == 0), stop=(dmb == 1),
                )
            szc = core.tile([128, T], F16, tag="szc", name="szc")
            if SILU_NATIVE:
                nc.scalar.activation(out=szc[:], in_=pz[:], func=ACT.Silu)
            else:
                sg2 = core.tile([128, T], F32, tag="sg2", name="sg2")
                nc.scalar.activation(out=sg2[:], in_=pz[:], func=ACT.Sigmoid)
                nc.vector.tensor_mul(szc[:], pz[:], sg2[:])
            dma(szd[b][:, t0:t0 + T], szc[:])

    # ---- phase 2: dbl (dt/B/C) and delta ----
    for c in range(nchunk):
        t0 = c * T
        pd = ps.tile([DT_RANK + 2 * NSTATE, T], F32, tag="pd", name="pd")
        for b in range(NBLK):
            nc.tensor.matmul(
                pd[:], wxp[b][:], xc[b][:, t0:t0 + T],
                start=(b == 0), stop=(b == NBLK - 1),
            )
        dbl = core.tile([DT_RANK + 2 * NSTATE, T], F16, tag="dbl", name="dbl")
        nc.scalar.activation(out=dbl[:], in_=pd[:], func=ACT.Copy)
        dma(bc[:, t0:t0 + T], dbl[DT_RANK:, :])
        for b in range(NBLK):
            pt = ps.tile([128, T], F32, tag="pt", name="pt")
            nc.tensor.matmul(
                pt[:], wdt[:, b * 128:(b + 1) * 128], dbl[0:DT_RANK, :],
                start=True, stop=True,
            )
            if SOFTPLUS_NATIVE:
                nc.scalar.activation(out=dl[b][:, t0:t0 + T], in_=pt[:],
                                     func=ACT.Softplus, bias=dtb[b][:], scale=1.0)
            else:
                et = core.tile([128, T], F32, tag="et", name="et")
                nc.scalar.activation(out=et[:], in_=pt[:], func=ACT.Exp,
                                     bias=dtb[b][:], scale=1.0)
                nc.scalar.activation(out=dl[b][:, t0:t0 + T], in_=et[:],
                                     func=ACT.Ln, bias=1.0, scale=1.0)

    # ---- phase 3: selective scan + gating + out-proj ----
    # L is processed in halves; within a half, each n-group's B/C rows are
    # broadcast once (via the DRAM bounce, partition-stride-0 read) and all
    # blocks/chunks consume them; per-group partial y sums accumulate into
    # yacc (g == 0 writes, g > 0 adds).
    NHL = 2 if L >= 2 * T else 1
    HL = L // NHL
    NCH = HL // T
    bc_ap = bc[:]
    for lh in range(NHL):
        h0 = lh * HL
        wh = []
        for b in range(NBLK):
            t = hp.tile([128, HL], F16, tag=f"wh{b}", name=f"wh{b}")
            nc.vector.tensor_mul(t[:], dl[b][:, h0:h0 + HL], xc[b][:, h0:h0 + HL])
            wh.append(t)
        yacc = [hp.tile([128, HL], F16, tag=f"ya{b}", name=f"ya{b}")
                for b in range(NBLK)]
        for g in range(NG):
            Bb = hp.tile([128, NJ, HL], F16, tag="Bb", name="Bb")
            dma(Bb, bass.AP(
                tensor=bc_ap.tensor,
                offset=bc_ap.offset + (g * NJ) * L + h0,
                ap=[[0, 128], [L, NJ], [1, HL]],
            ))
            Cb = hp.tile([128, NJ, HL], F16, tag="Cb", name="Cb")
            dma(Cb, bass.AP(
                tensor=bc_ap.tensor,
                offset=bc_ap.offset + (NSTATE + g * NJ) * L + h0,
                ap=[[0, 128], [L, NJ], [1, HL]],
            ))
            for ci in range(NCH):
                c = lh * NCH + ci
                s0 = ci * T
                for b in range(NBLK):
                    wh_ap = wh[b][:]
                    wrep = bass.AP(
                        tensor=wh_ap.tensor, offset=wh_ap.offset + s0,
                        ap=[wh_ap.ap[0], [0, NJ], [1, T]],
                    )
                    at = core.tile([128, NJ, T], F16, tag="at", name="at")
                    for j in range(NJ):
                        n = g * NJ + j
                        nc.scalar.activation(
                            out=at[:, j, :], in_=dl[b][:, h0 + s0:h0 + s0 + T],
                            func=ACT.Exp, scale=At[b][:, n:n + 1],
                        )
                    bt = core.tile([128, NJ, T], F16, tag="bt", name="bt", bufs=1)
                    nc.vector.tensor_mul(bt[:], wrep, Bb[:, :, s0:s0 + T])
                    ht = core.tile([128, NJ, T], F16, tag="ht", name="ht")
                    for j in range(NJ):
                        n = g * NJ + j
                        init = 0.0 if c == 0 else state[b][:, n:n + 1]
                        nc.vector.tensor_tensor_scan(
                            ht[:, j, :], at[:, j, :], bt[:, j, :], init,
                            AOP.mult, AOP.add,
                        )
                    nc.vector.tensor_copy(
                        out=state[b][:, g * NJ:(g + 1) * NJ], in_=ht[:, :, T - 1],
                    )
                    pt2 = core.tile([128, NJ, T], F16, tag="pt2", name="pt2", bufs=1)
                    nc.vector.tensor_mul(pt2[:], ht[:], Cb[:, :, s0:s0 + T])
                    q1 = core.tile([128, 2, T], F16, tag="q1", name="q1")
                    nc.vector.tensor_add(q1[:], pt2[:, 0:NJ:2, :], pt2[:, 1:NJ:2, :])
                    if g == 0:
                        nc.vector.tensor_add(
                            yacc[b][:, s0:s0 + T], q1[:, 0, :], q1[:, 1, :])
                    else:
                        qr = core.tile([128, T], F16, tag="qr", name="qr")
                        nc.vector.tensor_add(qr[:], q1[:, 0, :], q1[:, 1, :])
                        nc.vector.tensor_add(
                            yacc[b][:, s0:s0 + T], yacc[b][:, s0:s0 + T], qr[:])
        # gating + out-proj for this half
        for ci in range(NCH):
            c = lh * NCH + ci
            t0 = c * T
            s0 = ci * T
            gts = []
            for b in range(NBLK):
                szc2 = core.tile([128, T], F16, tag="szc2", name="szc2")
                dma(szc2, szd[b][:, t0:t0 + T])
                xd = core.tile([128, T], F16, tag="xd", name="xd")
                nc.scalar.activation(out=xd[:], in_=xc[b][:, t0:t0 + T],
                                     func=ACT.Copy, scale=Dd[b][:])
                y1 = core.tile([128, T], F16, tag="y1", name="y1")
                nc.vector.tensor_add(y1[:], xd[:], yacc[b][:, s0:s0 + T])
                gt = core.tile([128, T], F16, tag=f"gt{b}", name=f"gt{b}")
                nc.vector.tensor_mul(gt[:], y1[:], szc2[:])
                gts.append(gt)
            for mt in range(NMT):
                po = ps.tile([128, T], F32, tag="po", name="po", bufs=2)
                for b in range(NBLK):
                    nc.tensor.matmul(
                        po[:], wo[b][:, mt * 128:(mt + 1) * 128], gts[b][:],
                        start=(b == 0), stop=(b == NBLK - 1),
                    )
                if dirn == "b":
                    obs = core.tile([128, T], F16, tag="obs", name="obs")
                    nc.scalar.activation(out=obs[:], in_=po[:], func=ACT.Copy)
                    dma(ob[mt][:, t0:t0 + T], obs[:])
                else:
                    cb_rev = (L // T) - 1 - c
                    obs = core.tile([128, T], F16, tag="obs", name="obs")
                    dma(obs, ob[mt][:, cb_rev * T:(cb_rev + 1) * T])
                    oo = core.tile([128, T], F32, tag="oo", name="oo")
                    nc.vector.tensor_add(oo[:], po[:], obs[:, ::-1])
                    dma(y_param[mt * 128:(mt + 1) * 128, t0:t0 + T], oo[:])


def build_nc(L, T):
    _patch_act_tables()
    nc = bacc.Bacc("TRN2", target_bir_lowering=False, debug=False)
    prm = {}
    prm["u_f"] = nc.declare_dram_parameter("u_f", [D_MODEL, L + KCONV - 1], F16, isOutput=False)
    prm["u_b"] = nc.declare_dram_parameter("u_b", [D_MODEL, L + KCONV - 1], F16, isOutput=False)
    for d in ("f", "b"):
        prm[f"{d}_wx"] = nc.declare_dram_parameter(f"{d}_wx", [2 * KCONV, 128, D_INNER], F16, isOutput=False)
        prm[f"{d}_wz"] = nc.declare_dram_parameter(f"{d}_wz", [2, 128, D_INNER], F16, isOutput=False)
        prm[f"{d}_wxp"] = nc.declare_dram_parameter(f"{d}_wxp", [NBLK, 128, DT_RANK + 2 * NSTATE], F16, isOutput=False)
        prm[f"{d}_wdt"] = nc.declare_dram_parameter(f"{d}_wdt", [DT_RANK, D_INNER], F16, isOutput=False)
        prm[f"{d}_wo"] = nc.declare_dram_parameter(f"{d}_wo", [NBLK, 128, D_MODEL], F16, isOutput=False)
        prm[f"{d}_cb"] = nc.declare_dram_parameter(f"{d}_cb", [NBLK, 128, 1], F32, isOutput=False)
        prm[f"{d}_dtb"] = nc.declare_dram_parameter(f"{d}_dtb", [NBLK, 128, 1], F32, isOutput=False)
        prm[f"{d}_A"] = nc.declare_dram_parameter(f"{d}_A", [NBLK, 128, NSTATE], F32, isOutput=False)
        prm[f"{d}_D"] = nc.declare_dram_parameter(f"{d}_D", [NBLK, 128, 1], F32, isOutput=False)
    y_param = nc.declare_dram_parameter("y", [D_MODEL, L], F32, isOutput=True)

    with contextlib.ExitStack() as ctx:
        tc = ctx.enter_context(tile.TileContext(nc))
        pools = {
            "weights": ctx.enter_context(tc.tile_pool(name="weights", bufs=1)),
            "fullL": ctx.enter_context(tc.tile_pool(name="fullL", bufs=1)),
            "half": ctx.enter_context(tc.tile_pool(name="half", bufs=1)),
            "core": ctx.enter_context(tc.tile_pool(name="core", bufs=2)),
            "psum": ctx.enter_context(tc.tile_pool(name="psum", bufs=1, space="PSUM")),
            "dram": ctx.enter_context(tc.tile_pool(name="dram", bufs=1, space="DRAM")),
            "small": ctx.enter_context(tc.tile_pool(name="small", bufs=1)),
        }
        ob = [pools["dram"].tile([128, L], F16, tag=f"ob{m}", name=f"ob{m}") for m in range(NMT)]
        for dirn in ("b", "f"):
            _emit_dir(nc, pools, prm, dirn, L, T, ob, y_param[:])
    nc.compile()
    return nc


@functools.lru_cache(maxsize=None)
def _get_nc(L, T):
    return build_nc(L, T)


def _prep_dir_weights(pfx, inputs):
    in_w = np.asarray(inputs[pfx + "_in_w"], np.float32)      # [1024, 256]
    conv_w = np.asarray(inputs[pfx + "_conv_w"], np.float32)  # [512, 4]
    conv_b = np.asarray(inputs[pfx + "_conv_b"], np.float32)  # [512]
    xproj_w = np.asarray(inputs[pfx + "_xproj_w"], np.float32)  # [48, 512]
    dt_w = np.asarray(inputs[pfx + "_dt_w"], np.float32)      # [512, 16]
    dt_b = np.asarray(inputs[pfx + "_dt_b"], np.float32)      # [512]
    A_log = np.asarray(inputs[pfx + "_A_log"], np.float32)    # [512, 16]
    Dp = np.asarray(inputs[pfx + "_D"], np.float32)           # [512]
    out_w = np.asarray(inputs[pfx + "_out_w"], np.float32)    # [256, 512]

    in_w_x = in_w[:D_INNER]        # [512, 256]
    in_w_z = in_w[D_INNER:]        # [512, 256]
    wx = np.zeros([2 * KCONV, 128, D_INNER], np.float16)
    for k in range(KCONV):
        for dmb in range(2):
            # lhsT[(k,dmb) block][r, d] = in_w_x[d, dmb*128+r] * conv_w[d, k]
            wx[k * 2 + dmb] = (
                in_w_x[:, dmb * 128:(dmb + 1) * 128] * conv_w[:, k:k + 1]
            ).T.astype(np.float16)
    wz = np.stack([
        in_w_z[:, dmb * 128:(dmb + 1) * 128].T for dmb in range(2)
    ]).astype(np.float16)          # [2, 128, 512]
    wxp = np.stack([
        xproj_w[:, b * 128:(b + 1) * 128].T for b in range(NBLK)
    ]).astype(np.float16)          # [4, 128, 48]
    wdt = dt_w.T.astype(np.float16)  # [16, 512]
    wo = np.stack([
        0.5 * out_w[:, b * 128:(b + 1) * 128].T for b in range(NBLK)
    ]).astype(np.float16)          # [4, 128, 256]
    A = (-np.exp(A_log)).astype(np.float32)
    return {
        f"{pfx}_wx": wx,
        f"{pfx}_wz": wz,
        f"{pfx}_wxp": wxp,
        f"{pfx}_wdt": wdt,
        f"{pfx}_wo": wo,
        f"{pfx}_cb": conv_b.reshape(NBLK, 128, 1).astype(np.float32),
        f"{pfx}_dtb": dt_b.reshape(NBLK, 128, 1).astype(np.float32),
        f"{pfx}_A": A.reshape(NBLK, 128, NSTATE),
        f"{pfx}_D": Dp.reshape(NBLK, 128, 1).astype(np.float32),
    }


def make_in_maps(inputs, L):
    hs = np.asarray(inputs["hidden_states"], np.float32)  # [B, L, 256]
    B = hs.shape[0]
    wmap = {}
    for pfx in ("f", "b"):
        wmap.update(_prep_dir_weights(pfx, inputs))
    in_maps = []
    for c in range(B):
        u = np.ascontiguousarray(hs[c].T)  # [256, L]
        pad = np.zeros([D_MODEL, KCONV - 1], np.float32)
        u_f = np.concatenate([pad, u], axis=1).astype(np.float16)
        u_b = np.concatenate([pad, u[:, ::-1]], axis=1).astype(np.float16)
        m = dict(wmap)
        m["u_f"] = u_f
        m["u_b"] = u_b
        in_maps.append(m)
    return in_maps


def run(inputs, trace=False, **kwargs):
    from concourse.bass_utils import run_bass_kernel_spmd

    hs = np.asarray(inputs["hidden_states"], np.float32)
    B, L, _ = hs.shape
    nc = _get_nc(L, T_FULL if L % T_FULL == 0 else L)
    in_maps = make_in_maps(inputs, L)
    res = run_bass_kernel_spmd(nc, in_maps, list(range(N_CORES))[:B],
                               trace=trace, **kwargs)
    out = np.stack([
        np.asarray(res.results[c]["y"]).T for c in range(B)
    ]).astype(np.float32)
    return out, res


def kernel(**inputs):
    return run(inputs)[0]


# revision 18
# speedup vs baseline: 1.6503x; 1.0016x over previous
"""Bidirectional Mamba block on 8 Trainium2 NeuronCores.

Strategy
--------
Data-parallel over batch: each of the 8 cores runs one batch element
(both directions) end to end; no collectives.

Per core / per direction, with d_inner=512 split into 4 blocks of 128
partitions and the state dim n=16 split into 4 groups of 4:

  phase 1: x_conv = silu(conv1d(u @ in_w_x.T) + conv_b)  -- the depthwise
           causal conv is folded into the input projection on the PE by
           expanding the contraction dim to (4 taps x 256 d_model) with
           host-combined weights;  sz = silu(u @ in_w_z.T).
  phase 2: dbl = x_conv @ xproj.T (PE), delta = softplus(dt-proj + dt_b)
           (PE + one ACT Softplus pass). B/C rows of dbl bounce through a
           DRAM scratch so a stride-0-partition DMA can broadcast them to
           all 128 partitions.
  phase 3: per (block, n-group): a = exp(A[:,n] * delta) on ACT (scale is
           a per-partition AP, so the multiply is free), b = w * B_bcast,
           h = tensor_tensor_scan(a, b) (the native DVE linear-recurrence
           instruction, chained across t-chunks via per-partition initial
           values), y = sum_n h * C_bcast via a small add tree, then
           y = (y + x_conv * D) * sz, and the output projection on the PE
           (0.5 factor folded into out_w host-side).

The backward direction runs first on a host-reversed copy of the input;
its output (y_T) is stored to DRAM and combined, reversed, with the
forward direction's output projection.
"""

import contextlib
import functools
import sys

for _p in ("/opt/trn_rl_repo",):
    if _p not in sys.path:
        sys.path.insert(0, _p)

import numpy as np

import concourse.bass as bass
import concourse.bacc as bacc
import concourse.mybir as mybir
import concourse.tile as tile

F16 = mybir.dt.float16
F32 = mybir.dt.float32
AOP = mybir.AluOpType
ACT = mybir.ActivationFunctionType

D_MODEL = 256
D_INNER = 512
NSTATE = 16
DT_RANK = 16
KCONV = 4
NBLK = D_INNER // 128   # 4 d_inner blocks
NMT = D_MODEL // 128    # 2 d_model tiles
NJ = 4                  # n per group
NG = NSTATE // NJ       # 4 groups
N_CORES = 8

L_FULL = 4096
T_FULL = 512

# Native Silu ACT function exists on hardware but is not implemented by
# CoreSim; sim_test flips this to False to use sigmoid+multiply instead.
SILU_NATIVE = True
SOFTPLUS_NATIVE = False  # no ACT table provides Softplus on this toolchain


def _patch_act_tables():
    """Keep Exp and Ln in one table set (natural_log_exp_and_others) so the
    softplus (Exp+Ln) and the a-gen Exps never force ACT table reloads.
    Entries are blanked in place (positions preserved) because the emitted
    act_func_set_id indexes act_info.json by position."""
    import concourse.bacc as _bacc
    import concourse.hw_specs as _hw

    if getattr(_bacc, "_mamba_act_patch", False):
        return
    real = _hw.get_activation_tables

    def patched(arch):
        tabs = dict(real(arch))
        for nm in ("exp_and_others", "exp_and_friends", "natural_log"):
            if nm in tabs:
                tabs[nm] = set()
        return tabs

    _bacc.get_activation_tables = patched
    _bacc._mamba_act_patch = True


def _emit_dir(nc, pools, prm, W, dirn, lh, NHL, L, T, ob, y_param):
    HL = L // NHL
    h0 = lh * HL
    c0 = lh * (HL // T)
    nchunk_h = HL // T
    wp, fl, hp, core, ps, dram, sm = (
        pools["weights"], pools["fullL"], pools["half"], pools["core"],
        pools["psum"], pools["dram"], pools["small"],
    )

    def dma(out, in_):
        nc.sync.dma_start(out=out, in_=in_)

    wx, wz, wxp, wdt, wo, cb, dtb, At, Dd, state = (
        W['wx'], W['wz'], W['wxp'], W['wdt'], W['wo'], W['cb'], W['dtb'],
        W['A'], W['D'], W['state'])

    u_param = prm[f"u_{dirn}"]

    # ---- per-(direction, half) tensors; bufs=2 pipelines consecutive units ----
    xc = [hp.tile([128, HL], F16, tag=f"xc{b}", name=f"xc{b}", bufs=2) for b in range(NBLK)]
    dl = [hp.tile([128, HL], F16, tag=f"dl{b}", name=f"dl{b}", bufs=2) for b in range(NBLK)]
    szd = [dram.tile([128, HL], F16, tag=f"szd{b}", name=f"szd{b}", bufs=2) for b in range(NBLK)]
    bc = dram.tile([2 * NSTATE, HL], F16, tag="bc", name="bc", bufs=2)  # [32, HL]

    # ---- phase 1: x_conv (conv folded into in-proj) and sz ----
    for ci in range(nchunk_h):
        t0 = (c0 + ci) * T
        s0 = ci * T
        u0 = core.tile([128, T + KCONV - 1], F16, tag="u0", name="u0")
        u1 = core.tile([128, T + KCONV - 1], F16, tag="u1", name="u1")
        dma(u0, u_param[0:128, t0:t0 + T + KCONV - 1])
        dma(u1, u_param[128:256, t0:t0 + T + KCONV - 1])
        uu = (u0, u1)
        for b in range(NBLK):
            px = ps.tile([128, T], F32, tag="px", name="px", bufs=2)
            for kb in range(2 * KCONV):
                k, dmb = divmod(kb, 2)
                nc.tensor.matmul(
                    px[:], wx[kb][:, b * 128:(b + 1) * 128],
                    uu[dmb][:, k:k + T],
                    start=(kb == 0), stop=(kb == 2 * KCONV - 1),
                )
            if SILU_NATIVE:
                nc.scalar.activation(
                    out=xc[b][:, s0:s0 + T], in_=px[:], func=ACT.Silu,
                    bias=cb[b][:], scale=1.0,
                )
            else:
                xq = core.tile([128, T], F32, tag="xq", name="xq")
                nc.vector.tensor_scalar(out=xq[:], in0=px[:], scalar1=cb[b][:],
                                        scalar2=None, op0=AOP.add)
                sg = core.tile([128, T], F32, tag="sg", name="sg")
                nc.scalar.activation(out=sg[:], in_=xq[:], func=ACT.Sigmoid)
                nc.vector.tensor_mul(xc[b][:, s0:s0 + T], xq[:], sg[:])
        for b in range(NBLK):
            pz = ps.tile([128, T], F32, tag="pz", name="pz")
            for dmb in range(2):
                nc.tensor.matmul(
                    pz[:], wz[dmb][:, b * 128:(b + 1) * 128],
                    uu[dmb][:, KCONV - 1:KCONV - 1 + T],
                    start=(dmb == 0), stop=(dmb == 1),
                )
            szc = core.tile([128, T], F16, tag="szc", name="szc")
            if SILU_NATIVE:
                nc.scalar.activation(out=szc[:], in_=pz[:], func=ACT.Silu)
            else:
                sg2 = core.tile([128, T], F32, tag="sg2", name="sg2")
                nc.scalar.activation(out=sg2[:], in_=pz[:], func=ACT.Sigmoid)
                nc.vector.tensor_mul(szc[:], pz[:], sg2[:])
            dma(szd[b][:, s0:s0 + T], szc[:])

    # ---- phase 2: dbl (dt/B/C) and delta ----
    for ci in range(nchunk_h):
        s0 = ci * T
        pd = ps.tile([DT_RANK + 2 * NSTATE, T], F32, tag="pd", name="pd")
        for b in range(NBLK):
            nc.tensor.matmul(
                pd[:], wxp[b][:], xc[b][:, s0:s0 + T],
                start=(b == 0), stop=(b == NBLK - 1),
            )
        dbl = core.tile([DT_RANK + 2 * NSTATE, T], F16, tag="dbl", name="dbl")
        nc.scalar.activation(out=dbl[:], in_=pd[:], func=ACT.Copy)
        dma(bc[:, s0:s0 + T], dbl[DT_RANK:, :])
        for b in range(NBLK):
            pt = ps.tile([128, T], F32, tag="pt", name="pt")
            nc.tensor.matmul(
                pt[:], wdt[:, b * 128:(b + 1) * 128], dbl[0:DT_RANK, :],
                start=True, stop=True,
            )
            if SOFTPLUS_NATIVE:
                nc.scalar.activation(out=dl[b][:, s0:s0 + T], in_=pt[:],
                                     func=ACT.Softplus, bias=dtb[b][:], scale=1.0)
            else:
                et = core.tile([128, T], F32, tag="et", name="et")
                nc.scalar.activation(out=et[:], in_=pt[:], func=ACT.Exp,
                                     bias=dtb[b][:], scale=1.0)
                nc.scalar.activation(out=dl[b][:, s0:s0 + T], in_=et[:],
                                     func=ACT.Ln, bias=1.0, scale=1.0)

    # ---- phase 3: selective scan + gating + out-proj (this half) ----
    bc_ap = bc[:]
    wh = []
    for b in range(NBLK):
        t = hp.tile([128, HL], F16, tag=f"wh{b}", name=f"wh{b}")
        nc.vector.tensor_mul(t[:], dl[b][:], xc[b][:])
        wh.append(t)
    yacc = [hp.tile([128, HL], F16, tag=f"ya{b}", name=f"ya{b}")
            for b in range(NBLK)]
    for g in range(NG):
        Bb = hp.tile([128, NJ, HL], F16, tag="Bb", name="Bb")
        dma(Bb, bass.AP(
            tensor=bc_ap.tensor,
            offset=bc_ap.offset + (g * NJ) * HL,
            ap=[[0, 128], [HL, NJ], [1, HL]],
        ))
        Cb = hp.tile([128, NJ, HL], F16, tag="Cb", name="Cb")
        dma(Cb, bass.AP(
            tensor=bc_ap.tensor,
            offset=bc_ap.offset + (NSTATE + g * NJ) * HL,
            ap=[[0, 128], [HL, NJ], [1, HL]],
        ))
        for ci in range(nchunk_h):
            c = c0 + ci
            s0 = ci * T
            for b in range(NBLK):
                wh_ap = wh[b][:]
                wrep = bass.AP(
                    tensor=wh_ap.tensor, offset=wh_ap.offset + s0,
                    ap=[wh_ap.ap[0], [0, NJ], [1, T]],
                )
                at = core.tile([128, NJ, T], F16, tag="at", name="at")
                for j in range(NJ):
                    n = g * NJ + j
                    nc.scalar.activation(
                        out=at[:, j, :], in_=dl[b][:, s0:s0 + T],
                        func=ACT.Exp, scale=At[b][:, n:n + 1],
                    )
                bt = core.tile([128, NJ, T], F16, tag="bt", name="bt", bufs=1)
                nc.vector.tensor_mul(bt[:], wrep, Bb[:, :, s0:s0 + T])
                ht = core.tile([128, NJ, T], F16, tag="ht", name="ht")
                for j in range(NJ):
                    n = g * NJ + j
                    init = 0.0 if c == 0 else state[b][:, n:n + 1]
                    nc.vector.tensor_tensor_scan(
                        ht[:, j, :], at[:, j, :], bt[:, j, :], init,
                        AOP.mult, AOP.add,
                    )
                nc.scalar.activation(
                    out=state[b][:, g * NJ:(g + 1) * NJ], in_=ht[:, :, T - 1],
                    func=ACT.Copy,
                )
                pt2 = core.tile([128, NJ, T], F16, tag="pt2", name="pt2", bufs=1)
                nc.vector.tensor_mul(pt2[:], ht[:], Cb[:, :, s0:s0 + T])
                q1 = core.tile([128, 2, T], F16, tag="q1", name="q1")
                nc.vector.tensor_add(q1[:], pt2[:, 0:NJ:2, :], pt2[:, 1:NJ:2, :])
                if g == 0:
                    nc.vector.tensor_add(
                        yacc[b][:, s0:s0 + T], q1[:, 0, :], q1[:, 1, :])
                else:
                    qr = core.tile([128, T], F16, tag="qr", name="qr")
                    nc.vector.tensor_add(qr[:], q1[:, 0, :], q1[:, 1, :])
                    nc.vector.tensor_add(
                        yacc[b][:, s0:s0 + T], yacc[b][:, s0:s0 + T], qr[:])
    # gating + out-proj for this half
    for ci in range(nchunk_h):
        c = c0 + ci
        t0 = c * T
        s0 = ci * T
        gts = []
        for b in range(NBLK):
            szc2 = core.tile([128, T], F16, tag="szc2", name="szc2")
            dma(szc2, szd[b][:, s0:s0 + T])
            xd = core.tile([128, T], F16, tag="xd", name="xd")
            nc.scalar.activation(out=xd[:], in_=xc[b][:, s0:s0 + T],
                                 func=ACT.Copy, scale=Dd[b][:])
            y1 = core.tile([128, T], F16, tag="y1", name="y1")
            nc.vector.tensor_add(y1[:], xd[:], yacc[b][:, s0:s0 + T])
            gt = core.tile([128, T], F16, tag=f"gt{b}", name=f"gt{b}")
            nc.vector.tensor_mul(gt[:], y1[:], szc2[:])
            gts.append(gt)
        for mt in range(NMT):
            po = ps.tile([128, T], F32, tag="po", name="po", bufs=2)
            for b in range(NBLK):
                nc.tensor.matmul(
                    po[:], wo[b][:, mt * 128:(mt + 1) * 128], gts[b][:],
                    start=(b == 0), stop=(b == NBLK - 1),
                )
            if dirn == "b":
                obs = core.tile([128, T], F16, tag="obs", name="obs")
                nc.scalar.activation(out=obs[:], in_=po[:], func=ACT.Copy)
                dma(ob[mt][:, t0:t0 + T], obs[:])
            else:
                cb_rev = (L // T) - 1 - c
                obs = core.tile([128, T], F16, tag="obs", name="obs")
                dma(obs, ob[mt][:, cb_rev * T:(cb_rev + 1) * T])
                oo = core.tile([128, T], F32, tag="oo", name="oo")
                nc.vector.tensor_add(oo[:], po[:], obs[:, ::-1])
                dma(y_param[mt * 128:(mt + 1) * 128, t0:t0 + T], oo[:])


def _load_weights(nc, wp, sm, prm, dirn):
    def dma(out, in_):
        nc.sync.dma_start(out=out, in_=in_)
    W = {}
    W["wx"] = []
    for i in range(2 * KCONV):
        t = wp.tile([128, D_INNER], F16, tag=f"wx{i}", name=f"wx{i}")
        dma(t, prm[f"{dirn}_wx"][i])
        W["wx"].append(t)
    W["wz"] = []
    for i in range(2):
        t = wp.tile([128, D_INNER], F16, tag=f"wz{i}", name=f"wz{i}")
        dma(t, prm[f"{dirn}_wz"][i])
        W["wz"].append(t)
    W["wxp"] = []
    for b in range(NBLK):
        t = wp.tile([128, DT_RANK + 2 * NSTATE], F16, tag=f"wxp{b}", name=f"wxp{b}")
        dma(t, prm[f"{dirn}_wxp"][b])
        W["wxp"].append(t)
    W["wdt"] = wp.tile([DT_RANK, D_INNER], F16, tag="wdt", name="wdt")
    dma(W["wdt"], prm[f"{dirn}_wdt"][:])
    W["wo"] = []
    for b in range(NBLK):
        t = wp.tile([128, D_MODEL], F16, tag=f"wo{b}", name=f"wo{b}")
        dma(t, prm[f"{dirn}_wo"][b])
        W["wo"].append(t)
    for key, pname, width in (("cb", "cb", 1), ("dtb", "dtb", 1),
                              ("A", "A", NSTATE), ("D", "D", 1)):
        W[key] = []
        for b in range(NBLK):
            t = wp.tile([128, width], F32, tag=f"{key}{b}", name=f"{key}{b}")
            dma(t, prm[f"{dirn}_{pname}"][b])
            W[key].append(t)
    W["state"] = [sm.tile([128, NSTATE], F32, tag=f"st{b}", name=f"st{b}")
                  for b in range(NBLK)]
    return W


def build_nc(L, T):
    _patch_act_tables()
    nc = bacc.Bacc("TRN2", target_bir_lowering=False, debug=False)
    prm = {}
    prm["u_f"] = nc.declare_dram_parameter("u_f", [D_MODEL, L + KCONV - 1], F16, isOutput=False)
    prm["u_b"] = nc.declare_dram_parameter("u_b", [D_MODEL, L + KCONV - 1], F16, isOutput=False)
    for d in ("f", "b"):
        prm[f"{d}_wx"] = nc.declare_dram_parameter(f"{d}_wx", [2 * KCONV, 128, D_INNER], F16, isOutput=False)
        prm[f"{d}_wz"] = nc.declare_dram_parameter(f"{d}_wz", [2, 128, D_INNER], F16, isOutput=False)
        prm[f"{d}_wxp"] = nc.declare_dram_parameter(f"{d}_wxp", [NBLK, 128, DT_RANK + 2 * NSTATE], F16, isOutput=False)
        prm[f"{d}_wdt"] = nc.declare_dram_parameter(f"{d}_wdt", [DT_RANK, D_INNER], F16, isOutput=False)
        prm[f"{d}_wo"] = nc.declare_dram_parameter(f"{d}_wo", [NBLK, 128, D_MODEL], F16, isOutput=False)
        prm[f"{d}_cb"] = nc.declare_dram_parameter(f"{d}_cb", [NBLK, 128, 1], F32, isOutput=False)
        prm[f"{d}_dtb"] = nc.declare_dram_parameter(f"{d}_dtb", [NBLK, 128, 1], F32, isOutput=False)
        prm[f"{d}_A"] = nc.declare_dram_parameter(f"{d}_A", [NBLK, 128, NSTATE], F32, isOutput=False)
        prm[f"{d}_D"] = nc.declare_dram_parameter(f"{d}_D", [NBLK, 128, 1], F32, isOutput=False)
    y_param = nc.declare_dram_parameter("y", [D_MODEL, L], F32, isOutput=True)

    with contextlib.ExitStack() as ctx:
        tc = ctx.enter_context(tile.TileContext(nc))
        pools = {
            "weights": ctx.enter_context(tc.tile_pool(name="weights", bufs=1)),
            "fullL": ctx.enter_context(tc.tile_pool(name="fullL", bufs=1)),
            "half": ctx.enter_context(tc.tile_pool(name="half", bufs=1)),
            "core": ctx.enter_context(tc.tile_pool(name="core", bufs=2)),
            "psum": ctx.enter_context(tc.tile_pool(name="psum", bufs=1, space="PSUM")),
            "dram": ctx.enter_context(tc.tile_pool(name="dram", bufs=1, space="DRAM")),
            "small": ctx.enter_context(tc.tile_pool(name="small", bufs=1)),
        }
        ob = [pools["dram"].tile([128, L], F16, tag=f"ob{m}", name=f"ob{m}") for m in range(NMT)]
        NHL = 2 if L >= 2 * T else 1
        for dirn in ("b", "f"):
            W = _load_weights(nc, pools["weights"], pools["small"], prm, dirn)
            for lh in range(NHL):
                _emit_dir(nc, pools, prm, W, dirn, lh, NHL, L, T, ob, y_param[:])
    nc.compile()
    return nc


@functools.lru_cache(maxsize=None)
def _get_nc(L, T):
    return build_nc(L, T)


def _prep_dir_weights(pfx, inputs):
    in_w = np.asarray(inputs[pfx + "_in_w"], np.float32)      # [1024, 256]
    conv_w = np.asarray(inputs[pfx + "_conv_w"], np.float32)  # [512, 4]
    conv_b = np.asarray(inputs[pfx + "_conv_b"], np.float32)  # [512]
    xproj_w = np.asarray(inputs[pfx + "_xproj_w"], np.float32)  # [48, 512]
    dt_w = np.asarray(inputs[pfx + "_dt_w"], np.float32)      # [512, 16]
    dt_b = np.asarray(inputs[pfx + "_dt_b"], np.float32)      # [512]
    A_log = np.asarray(inputs[pfx + "_A_log"], np.float32)    # [512, 16]
    Dp = np.asarray(inputs[pfx + "_D"], np.float32)           # [512]
    out_w = np.asarray(inputs[pfx + "_out_w"], np.float32)    # [256, 512]

    in_w_x = in_w[:D_INNER]        # [512, 256]
    in_w_z = in_w[D_INNER:]        # [512, 256]
    wx = np.zeros([2 * KCONV, 128, D_INNER], np.float16)
    for k in range(KCONV):
        for dmb in range(2):
            # lhsT[(k,dmb) block][r, d] = in_w_x[d, dmb*128+r] * conv_w[d, k]
            wx[k * 2 + dmb] = (
                in_w_x[:, dmb * 128:(dmb + 1) * 128] * conv_w[:, k:k + 1]
            ).T.astype(np.float16)
    wz = np.stack([
        in_w_z[:, dmb * 128:(dmb + 1) * 128].T for dmb in range(2)
    ]).astype(np.float16)          # [2, 128, 512]
    wxp = np.stack([
        xproj_w[:, b * 128:(b + 1) * 128].T for b in range(NBLK)
    ]).astype(np.float16)          # [4, 128, 48]
    wdt = dt_w.T.astype(np.float16)  # [16, 512]
    wo = np.stack([
        0.5 * out_w[:, b * 128:(b + 1) * 128].T for b in range(NBLK)
    ]).astype(np.float16)          # [4, 128, 256]
    A = (-np.exp(A_log)).astype(np.float32)
    return {
        f"{pfx}_wx": wx,
        f"{pfx}_wz": wz,
        f"{pfx}_wxp": wxp,
        f"{pfx}_wdt": wdt,
        f"{pfx}_wo": wo,
        f"{pfx}_cb": conv_b.reshape(NBLK, 128, 1).astype(np.float32),
        f"{pfx}_dtb": dt_b.reshape(NBLK, 128, 1).astype(np.float32),
        f"{pfx}_A": A.reshape(NBLK, 128, NSTATE),
        f"{pfx}_D": Dp.reshape(NBLK, 128, 1).astype(np.float32),
    }


def make_in_maps(inputs, L):
    hs = np.asarray(inputs["hidden_states"], np.float32)  # [B, L, 256]
    B = hs.shape[0]
    wmap = {}
    for pfx in ("f", "b"):
        wmap.update(_prep_dir_weights(pfx, inputs))
    in_maps = []
    for c in range(B):
        u = np.ascontiguousarray(hs[c].T)  # [256, L]
        pad = np.zeros([D_MODEL, KCONV - 1], np.float32)
        u_f = np.concatenate([pad, u], axis=1).astype(np.float16)
        u_b = np.concatenate([pad, u[:, ::-1]], axis=1).astype(np.float16)
        m = dict(wmap)
        m["u_f"] = u_f
        m["u_b"] = u_b
        in_maps.append(m)
    return in_maps


def run(inputs, trace=False, **kwargs):
    from concourse.bass_utils import run_bass_kernel_spmd

    hs = np.asarray(inputs["hidden_states"], np.float32)
    B, L, _ = hs.shape
    nc = _get_nc(L, T_FULL if L % T_FULL == 0 else L)
    in_maps = make_in_maps(inputs, L)
    res = run_bass_kernel_spmd(nc, in_maps, list(range(N_CORES))[:B],
                               trace=trace, **kwargs)
    out = np.stack([
        np.asarray(res.results[c]["y"]).T for c in range(B)
    ]).astype(np.float32)
    return out, res


def kernel(**inputs):
    return run(inputs)[0]
